# revision 29
# baseline (speedup 1.0000x reference)
"""DGCNN-style EdgeConv model on 8 Trainium2 NeuronCores.

Strategy: data-parallel over batch (1 point-cloud per core). Training-mode
batch-norm statistics are global over the batch axis, so each core computes
per-channel partial (sum, sumsq) and an 8-core AllReduce combines them
before the normalize+LeakyReLU is applied.

Key device-side tricks:
  - kNN top-20 via pd' = 2*x^T x - |x_m|^2 (row-rank-equivalent to the
    reference's pairwise-distance matrix), computed entirely on the PE by
    augmenting the contraction with a ones-row; exact top-24 per row via
    3 rounds of DVE max8/max_index/match_replace.
  - EdgeConv 1x1 convs commute with the neighbor gather:
    W*[x_m - x_n; x_n] = U[., m] + V[., n] with U = Wa*X, V = (Wb-Wa)*X,
    so the conv runs on N points instead of N*K gathered features, and the
    gather (gpsimd ap_gather) moves conv outputs.
  - conv7's 1024-channel global-feature block is a matrix-vector product
    (g is constant over points), folded in as a per-channel bias.
"""

import numpy as np

N, K, NCORES, EPS = 2048, 20, 8, 1e-5

MATMUL_WEIGHTS = ("w1a", "w1d", "w2", "w3a", "w3d", "w4", "w5a", "w5d",
                  "w6", "w7g", "w7x", "w8", "w9")


def round_f32r(a):
    """Round fp32 array to f32r (s1e8m11) with round-to-nearest-even."""
    u = np.ascontiguousarray(a, np.float32).view(np.uint32)
    lsb = (u >> 12) & 1
    u = (u + 0x7FF + lsb) & np.uint32(0xFFFFF000)
    return u.view(np.float32)


# --------------------------------------------------------------------------
# Host-side weight preprocessing (layout only; all heavy math on device)
# --------------------------------------------------------------------------
def prep_weights(params):
    p = {k: np.asarray(v, np.float32) for k, v in params.items()}
    w = {}
    cc = np.ascontiguousarray

    w1 = p["w1"]  # [64, 12]
    w["w1a"] = cc(w1[:, :6].T)                 # [6, 64]
    w["w1d"] = cc((w1[:, 6:] - w1[:, :6]).T)   # [6, 64]
    w["w2"] = cc(p["w2"].T)                    # [64, 64]
    w3 = p["w3"]  # [64, 128]
    w["w3a"] = cc(w3[:, :64].T)
    w["w3d"] = cc((w3[:, 64:] - w3[:, :64]).T)
    w["w4"] = cc(p["w4"].T)
    w5 = p["w5"]
    w["w5a"] = cc(w5[:, :64].T)
    w["w5d"] = cc((w5[:, 64:] - w5[:, :64]).T)
    # w6 [1024, 192] -> lhsT [192, 1024] -> [64, 3, 1024]
    w["w6"] = cc(p["w6"].T.reshape(3, 64, 1024).transpose(1, 0, 2))
    w7 = p["w7"]  # [512, 1216], input channels ordered [g(1024), x1, x2, x3]
    w["w7g"] = cc(w7[:, :1024].T.reshape(8, 128, 512).transpose(1, 0, 2))  # [128,8,512]
    w["w7x"] = cc(w7[:, 1024:].T.reshape(3, 64, 512).transpose(1, 0, 2))   # [64,3,512]
    w["w8"] = cc(p["w8"].T.reshape(4, 128, 256).transpose(1, 0, 2))        # [128,4,256]
    w["w9"] = cc(p["w9"].T.reshape(2, 128, 4).transpose(1, 0, 2))          # [128,2,4]

    for i in range(1, 6):
        w[f"g{i}"] = cc(p[f"g{i}"].reshape(64, 1))
        w[f"b{i}"] = cc(p[f"b{i}"].reshape(64, 1))
    w["g6"] = cc(p["g6"].reshape(8, 128).T)    # [128, 8]  channel = t*128 + p
    w["b6"] = cc(p["b6"].reshape(8, 128).T)
    w["g7"] = cc(p["g7"].reshape(4, 128).T)    # [128, 4]
    w["b7"] = cc(p["b7"].reshape(4, 128).T)
    w["g8"] = cc(p["g8"].reshape(2, 128).T)    # [128, 2]
    w["b8"] = cc(p["b8"].reshape(2, 128).T)
    return w


WEIGHT_SHAPES = {
    "w1a": (6, 64), "w1d": (6, 64), "w2": (64, 64),
    "w3a": (64, 64), "w3d": (64, 64), "w4": (64, 64),
    "w5a": (64, 64), "w5d": (64, 64),
    "w6": (64, 3, 1024), "w7g": (128, 8, 512), "w7x": (64, 3, 512),
    "w8": (128, 4, 256), "w9": (128, 2, 4),
    "g1": (64, 1), "b1": (64, 1), "g2": (64, 1), "b2": (64, 1),
    "g3": (64, 1), "b3": (64, 1), "g4": (64, 1), "b4": (64, 1),
    "g5": (64, 1), "b5": (64, 1),
    "g6": (128, 8), "b6": (128, 8), "g7": (128, 4), "b7": (128, 4),
    "g8": (128, 2), "b8": (128, 2),
}


# --------------------------------------------------------------------------
# Device kernel
# --------------------------------------------------------------------------
def model_body(tc, out_ap, ins, n=N, k=K, n_cores=NCORES, dbg=None):
    import concourse.bass as bass
    import concourse.mybir as mybir
    from contextlib import ExitStack

    nc = tc.nc
    f32 = mybir.dt.float32
    f32r = mybir.dt.float32r
    i16 = mybir.dt.int16
    u16 = mybir.dt.uint16
    AF = mybir.ActivationFunctionType
    ALU = mybir.AluOpType
    AX = mybir.AxisListType
    RG = [list(range(n_cores))]

    HALF = n // 2
    NB = n // 128          # knn row-blocks
    NBH = NB // 2          # row-blocks per half
    QH = HALF // 16        # q'-groups per half
    HK = HALF * k          # gathered positions per half (free size of slabs)
    NSLAB = 4
    SLABW = HK // NSLAB
    QSLAB = QH // NSLAB    # q'-groups per slab
    CNT2 = float(n_cores * n * k)   # bn1-5 element count
    CNT1 = float(n_cores * n)       # bn6-8 element count
    ALPHA = 0.2

    def r(ap):
        return ap

    def fb(ap):
        return ap

    def chunks(total, step=512):
        c0 = 0
        while c0 < total:
            yield c0, min(step, total - c0)
            c0 += step

    with ExitStack() as ctx:
        consts = ctx.enter_context(tc.tile_pool(name="consts", bufs=1))
        persist = ctx.enter_context(tc.tile_pool(name="persist", bufs=1))
        stat = ctx.enter_context(tc.tile_pool(name="stat", bufs=2))
        dram = ctx.enter_context(tc.tile_pool(name="dram", bufs=2, space="DRAM"))
        psum = ctx.enter_context(tc.tile_pool(name="psum", bufs=2, space="PSUM"))

        # ---- load small constants
        ones3 = consts.tile([3, 1], f32)
        nc.vector.memset(ones3, 1.0)
        epsc = consts.tile([128, 1], f32, tag="epsc")
        nc.vector.memset(epsc, EPS)

        def load_const(name):
            t = consts.tile(list(ins[name].shape), ins[name].dtype, tag=name)
            nc.sync.dma_start(out=t, in_=ins[name])
            return t

        xin = consts.tile([6, n], f32, tag="xin")
        nc.sync.dma_start(out=xin, in_=ins["x"])
        xr = xin
        wsb = {nm: load_const(nm) for nm in
               ["w1a", "w1d", "w3a", "w3d", "w5a", "w5d"]}
        # w2/w4 duplicated into both partition halves: the slab layout keeps
        # half-1 activations at partitions 64-127, and the PE contracts over
        # the partitions its operands live on.
        for nm in ["w2", "w4"]:
            t = consts.tile([128, 64], f32, tag=nm + "dup")
            nc.sync.dma_start(out=t[0:64, :], in_=ins[nm])
            nc.sync.dma_start(out=t[64:128, :], in_=ins[nm])
            wsb[nm] = t
        bn_sb = {nm: load_const(nm) for nm in
                 ["g1", "b1", "g2", "b2", "g3", "b3", "g4", "b4", "g5", "b5",
                  "g6", "b6", "g7", "b7", "g8", "b8"]}

        # x1/x2/x3 full-layout feature maps
        x1 = persist.tile([64, n], f32, tag="x1")
        x2 = persist.tile([64, n], f32, tag="x2")
        x3 = persist.tile([64, n], f32, tag="x3")

        # ------------------------------------------------------------------
        # helpers
        # ------------------------------------------------------------------
        def emit_knn(src, src_exact, knnpool, vpool, idxpool):
            """src: AP [*, n] with 3 coord rows at partitions 0..2.
            pd' = 2<x_n, x_m> - |x_m|^2 on the PE via an augmented ones-row;
            x is split hi/lo at f32r precision so the reduced-precision
            matmul mode still yields fp32-exact pairwise ranks.
            Returns idxw [128, QH*k] int16 wrapped neighbor indices."""
            lhs4 = small.tile([4, n], f32, tag="lhs4")
            rhs4 = small.tile([4, n], f32, tag="rhs4")
            # exact xx = sum_c src[c]^2 in fp32 (DVE adds across 3 rows)
            sq3 = slab.tile([3, n], f32, tag="slab")
            nc.scalar.activation(sq3, fb(src[0:3, :]), AF.Square)
            t1 = slab.tile([1, n], f32, tag="slab")
            t2 = slab.tile([1, n], f32, tag="slab")
            nc.sync.dma_start(out=t1, in_=sq3[1:2, :])
            nc.sync.dma_start(out=t2, in_=sq3[2:3, :])
            nc.vector.tensor_tensor(out=t2, in0=t2, in1=t1, op=ALU.add)
            nc.vector.tensor_tensor(out=t1, in0=t2, in1=sq3[0:1, :], op=ALU.add)
            nc.scalar.activation(t1, t1, AF.Copy, scale=-1.0)      # -xx
            # rhs4 = [x; -xx], lhs4 = [2x; ones]
            nc.scalar.activation(rhs4[0:3, :], src[0:3, :], AF.Copy)
            nc.sync.dma_start(out=rhs4[3:4, :], in_=t1)
            nc.vector.memset(lhs4, 1.0)
            nc.scalar.activation(lhs4[0:3, :], src[0:3, :], AF.Copy, scale=2.0)

            idxw = idxpool.tile([128, QH * k], i16, tag="idxw")
            stage = dram.tile([NB, 128, 24], u16, tag="idxstage")
            for b in range(NB):
                pd_ps = psum.tile([128, n], f32, tag="ps")
                bs = slice(b * 128, (b + 1) * 128)
                for c0, cw in chunks(n):
                    cs = slice(c0, c0 + cw)
                    nc.tensor.matmul(pd_ps[:, cs], lhsT=lhs4[:, bs],
                                     rhs=rhs4[:, cs])
                pdw = knnpool.tile([128, n], f32, tag="pdw")
                nc.scalar.activation(pdw, pd_ps, AF.Copy)
                vals = vpool.tile([128, 24], f32, tag="vals")
                idx = vpool.tile([128, 24], u16, tag="idx")
                for rd in range(3):
                    sl = slice(8 * rd, 8 * rd + 8)
                    nc.vector.max(out=vals[:, sl], in_=pdw)
                    nc.vector.max_index(out=idx[:, sl], in_max=vals[:, sl],
                                        in_values=pdw)
                    if rd < 2:
                        nc.vector.match_replace(out=pdw, in_to_replace=vals[:, sl],
                                                in_values=pdw, imm_value=-1e30)
                nc.sync.dma_start(out=stage[b], in_=idx)
            # Re-layout staged indices into ap_gather's wrapped order:
            # list position i = q'*(16k) + k2*16 + rr <-> (n_loc = q'*16 + rr, k2);
            # wrapped element i lives at [i % 16, i // 16] of each 16-partition
            # group. DRAM staging makes the permutation a flat strided AP.
            for h in range(2):
                src = stage[h * NBH:(h + 1) * NBH, :, 0:k].bitcast(i16).rearrange(
                    "b (q rr) k2 -> rr b q k2", rr=16)
                for g in range(4):
                    p0 = 64 * h + 16 * g
                    dst = idxw[p0:p0 + 16, :].rearrange(
                        "rr (b q k2) -> rr b q k2", b=NBH, k2=k)
                    nc.sync.dma_start(out=dst, in_=src)
            return idxw

        def emit_conv_uv(wa, wd, src, cin, Ud, Vd):
            """U = wa^T @ src duplicated into both partition halves of Ud
            [128, n]; V = wd^T @ src into half-layout Vd [128, HALF]."""
            psU = psum.tile([128, n], f32, tag="ps")
            for h2 in range(2):
                for c0, cw in chunks(n):
                    nc.tensor.matmul(psU[64 * h2:64 * h2 + 64, c0:c0 + cw],
                                     lhsT=r(wa), rhs=r(src[0:cin, c0:c0 + cw]))
            nc.scalar.activation(Ud, psU, AF.Copy)
            psV = psum.tile([128, n], f32, tag="ps")
            for h2 in range(2):
                for c0, cw in chunks(n):
                    nc.tensor.matmul(psV[64 * h2:64 * h2 + 64, c0:c0 + cw],
                                     lhsT=r(wd), rhs=r(src[0:cin, c0:c0 + cw]))
            nc.scalar.activation(Vd[0:64, :], psV[0:64, 0:HALF], AF.Copy)
            nc.scalar.activation(Vd[64:128, :], psV[64:128, HALF:n], AF.Copy)

        def vbcast(Vd, s):
            """V broadcast AP matching slab s's [p, q, k2, rr] layout."""
            v = Vd[:, s * QSLAB * 16:(s + 1) * QSLAB * 16].rearrange(
                "p (q rr) -> p q rr", rr=16)
            return bass.AP(tensor=v.tensor, offset=v.offset,
                           ap=[list(v.ap[0]), list(v.ap[1]), [0, k], list(v.ap[2])])

        def sq_stats(trash, trash_tag, src, width, s2ap):
            """s2ap[128,1] = sum(src[:, :width]^2) via chunked ACT Square."""
            ncc = len(list(chunks(width, 2048)))
            sub = stat.tile([128, ncc], f32, tag="sqsum")
            for ci, (c0, cw) in enumerate(chunks(width, 2048)):
                tr = trash.tile([128, 2048], f32, tag=trash_tag)
                nc.scalar.activation(tr[:, 0:cw], fb(src[:, c0:c0 + cw]),
                                     AF.Square, accum_out=sub[:, ci:ci + 1])
            nc.vector.tensor_reduce(out=s2ap, in_=sub, axis=AX.X, op=ALU.add)

        def emit_gather_pre(idxw, Ud, Vd, slab, trash, s1, s2):
            """hpre[s] = gather(Ud)[i] + V[n(i)]; accumulate sum/sumsq."""
            tiles = []
            for s in range(NSLAB):
                hp = slab.tile([128, SLABW], f32, tag="slab")
                nc.gpsimd.ap_gather(
                    out_ap=hp, in_ap=Ud,
                    idxs_ap=idxw[:, s * (SLABW // 16):(s + 1) * (SLABW // 16)],
                    channels=128, num_elems=n, d=1, num_idxs=SLABW)
                hp4 = hp.rearrange("p (q k2 rr) -> p q k2 rr", k2=k, rr=16)
                nc.vector.tensor_tensor(out=hp4, in0=hp4, in1=vbcast(Vd, s),
                                        op=ALU.add)
                nc.vector.tensor_scalar(out=hp, in0=hp, scalar1=1.0,
                                        scalar2=0.0, op0=ALU.mult,
                                        op1=ALU.add,
                                        accum_out=s1[:, s:s + 1])
                sq_stats(trash, "pdw", hp, SLABW, s2[:, s:s + 1])
                tiles.append(hp)
            return tiles

        def emit_conv_slab(w, hp_tiles, trash, s1, s2):
            """z[s] = w^T @ hp[s] per partition-half, evicted IN PLACE over
            the consumed hpre columns; accumulate stats."""
            for s in range(NSLAB):
                zt = hp_tiles[s]
                nch = len(list(chunks(SLABW)))
                sub1 = stat.tile([128, nch], f32, tag="csum")
                for ci, (c0, cw) in enumerate(chunks(SLABW)):
                    ps = psum.tile([128, 512], f32, tag="ps")
                    for h2 in range(2):
                        pr = slice(64 * h2, 64 * h2 + 64)
                        nc.tensor.matmul(
                            ps[pr, 0:cw], lhsT=r(w[pr, :]),
                            rhs=r(zt[pr, c0:c0 + cw]))
                    nc.scalar.activation(zt[:, c0:c0 + cw], ps[:, 0:cw], AF.Copy,
                                         accum_out=sub1[:, ci:ci + 1])
                nc.vector.tensor_reduce(out=s1[:, s:s + 1], in_=sub1,
                                        axis=AX.X, op=ALU.add)
                sq_stats(trash, "pdw", zt, SLABW, s2[:, s:s + 1])
            return hp_tiles

        def bn_reduce(s1, s2, F, cnt, g_sb, b_sb, fold):
            """AllReduce partial sums; return per-channel (a, b') affine.
            fold=True: [128,*] half-duplicated layout folded to 64 channels,
            result duplicated back to 128 partitions."""
            if fold:
                s1r = stat.tile([128, 1], f32, tag="s1r")
                s2r = stat.tile([128, 1], f32, tag="s2r")
                nc.vector.tensor_reduce(out=s1r, in_=s1, axis=AX.X, op=ALU.add)
                nc.vector.tensor_reduce(out=s2r, in_=s2, axis=AX.X, op=ALU.add)
                tmp = stat.tile([64, 2], f32, tag="pack2")
                nc.sync.dma_start(out=tmp[:, 0:1], in_=s1r[64:128, :])
                nc.sync.dma_start(out=tmp[:, 1:2], in_=s2r[64:128, :])
                pack = stat.tile([64, 2], f32, tag="pack")
                nc.vector.tensor_tensor(out=pack[:, 0:1], in0=s1r[0:64, :],
                                        in1=tmp[:, 0:1], op=ALU.add)
                nc.vector.tensor_tensor(out=pack[:, 1:2], in0=s2r[0:64, :],
                                        in1=tmp[:, 1:2], op=ALU.add)
                P = 64
            else:
                pack = stat.tile([128, 2 * F], f32, tag="packL")
                nc.vector.tensor_copy(out=pack[:, 0:F], in_=s1)
                nc.vector.tensor_copy(out=pack[:, F:2 * F], in_=s2)
                P = 128
            cc_in = dram.tile([P, 2 * F], f32, tag="ccin")
            cc_out = dram.tile([P, 2 * F], f32, tag="ccout")
            nc.sync.dma_start(out=cc_in, in_=pack)
            nc.gpsimd.collective_compute(
                "AllReduce", ALU.add, replica_groups=RG,
                ins=[cc_in[:, :].opt()], outs=[cc_out[:, :].opt()])
            red = stat.tile([P, 2 * F], f32, tag="red")
            nc.sync.dma_start(out=red, in_=cc_out)
            mm = stat.tile([P, 2 * F], f32, tag="mm")
            nc.vector.tensor_scalar(out=mm, in0=red, scalar1=1.0 / cnt,
                                    scalar2=None, op0=ALU.mult)
            var = stat.tile([P, F], f32, tag="var")
            nc.vector.tensor_tensor(out=var, in0=mm[:, 0:F], in1=mm[:, 0:F],
                                    op=ALU.mult)
            nc.vector.tensor_tensor(out=var, in0=mm[:, F:2 * F], in1=var,
                                    op=ALU.subtract)
            sig = stat.tile([P, F], f32, tag="sig")
            nc.scalar.activation(sig, var, AF.Sqrt, bias=epsc[0:P, :])
            rs = stat.tile([P, F], f32, tag="rs")
            nc.vector.reciprocal(out=rs, in_=sig)
            a0 = stat.tile([P, F], f32, tag="a0")
            bp0 = stat.tile([P, F], f32, tag="bp0")
            nc.vector.tensor_tensor(out=a0, in0=g_sb, in1=rs, op=ALU.mult)
            nc.vector.tensor_tensor(out=bp0, in0=mm[:, 0:F], in1=a0, op=ALU.mult)
            nc.vector.tensor_tensor(out=bp0, in0=b_sb, in1=bp0, op=ALU.subtract)
            if not fold:
                return a0, bp0
            a = stat.tile([128, 1], f32, tag="aD")
            bp = stat.tile([128, 1], f32, tag="bpD")
            nc.sync.dma_start(out=a[0:64, :], in_=a0)
            nc.sync.dma_start(out=a[64:128, :], in_=a0)
            nc.sync.dma_start(out=bp[0:64, :], in_=bp0)
            nc.sync.dma_start(out=bp[64:128, :], in_=bp0)
            return a, bp

        def bn_apply(tiles, a, bp, cols=None):
            """In-place y = lrelu(a*x + b') on each tile (or column slice)."""
            for t_i, t in enumerate(tiles):
                v = t if cols is None else t[:, cols[t_i][0]:cols[t_i][1]]
                av = a if a.shape[1] == 1 else a[:, t_i:t_i + 1]
                bv = bp if bp.shape[1] == 1 else bp[:, t_i:t_i + 1]
                nc.vector.tensor_scalar(out=v, in0=fb(v), scalar1=av,
                                        scalar2=bv, op0=ALU.mult, op1=ALU.add)
                nc.vector.scalar_tensor_tensor(out=v, in0=fb(v), scalar=ALPHA,
                                               in1=fb(v), op0=ALU.mult,
                                               op1=ALU.max)

        def emit_kmax(z_tiles, xfull, xdup_pool):
            """x[c, n] = max over k of z[c, i]; un-duplicate halves."""
            xdup = xdup_pool.tile([128, HALF], f32, tag="Vd")
            for s, zt in enumerate(z_tiles):
                zv = zt.rearrange("p (q k2 rr) -> p q rr k2", k2=k, rr=16)
                ov = xdup[:, s * QSLAB * 16:(s + 1) * QSLAB * 16].rearrange(
                    "p (q rr) -> p q rr", rr=16)
                nc.vector.tensor_reduce(out=ov, in_=zv, axis=AX.X, op=ALU.max)
            nc.sync.dma_start(out=xfull[:, 0:HALF], in_=xdup[0:64, :])
            nc.sync.dma_start(out=xfull[:, HALF:n], in_=xdup[64:128, :])

        # ------------------------------------------------------------------
        # Blocks A, B, C (EdgeConv stages)
        # ------------------------------------------------------------------
        with ExitStack() as abc:
            small = abc.enter_context(tc.tile_pool(name="small", bufs=1))
            slab = abc.enter_context(tc.tile_pool(name="slab", bufs=4))
            knnpool = abc.enter_context(tc.tile_pool(name="knn", bufs=2))
            trash = knnpool
            vpool = abc.enter_context(tc.tile_pool(name="vidx", bufs=3))
            idxpool = abc.enter_context(tc.tile_pool(name="idxw", bufs=1))
            uv = abc.enter_context(tc.tile_pool(name="uv", bufs=1))

            def edge_block(src_knn, src_exact, src_conv, cin, wa, wd,
                           w_second, gA, bA, gB, bB, xout):
                idxw = emit_knn(src_knn, src_exact, knnpool, vpool, idxpool)
                Ud = uv.tile([128, n], f32, tag="Ud")
                Vd = uv.tile([128, HALF], f32, tag="Vd")
                emit_conv_uv(wa, wd, src_conv, cin, Ud, Vd)
                s1 = stat.tile([128, NSLAB], f32, tag="s1")
                s2 = stat.tile([128, NSLAB], f32, tag="s2")
                hp_tiles = emit_gather_pre(idxw, Ud, Vd, slab, trash, s1, s2)
                a, bp = bn_reduce(s1, s2, 1, CNT2, bn_sb[gA], bn_sb[bA], True)
                bn_apply(hp_tiles, a, bp)
                if w_second is not None:
                    s1b = stat.tile([128, NSLAB], f32, tag="s1")
                    s2b = stat.tile([128, NSLAB], f32, tag="s2")
                    z_tiles = emit_conv_slab(w_second, hp_tiles, trash,
                                             s1b, s2b)
                    a2, bp2 = bn_reduce(s1b, s2b, 1, CNT2, bn_sb[gB],
                                        bn_sb[bB], True)
                    bn_apply(z_tiles, a2, bp2)
                else:
                    z_tiles = hp_tiles
                emit_kmax(z_tiles, xout, uv)

            edge_block(xin, True, xr, 6, wsb["w1a"], wsb["w1d"], wsb["w2"],
                       "g1", "b1", "g2", "b2", x1)
            edge_block(x1, False, x1, 64, wsb["w3a"], wsb["w3d"], wsb["w4"],
                       "g3", "b3", "g4", "b4", x2)
            edge_block(x2, False, x2, 64, wsb["w5a"], wsb["w5d"], None,
                       "g5", "b5", None, None, x3)
            if dbg is not None:
                for nm, t in [("x1", x1), ("x2", x2), ("x3", x3)]:
                    nc.sync.dma_start(out=dbg[nm], in_=fb(t))

        # ------------------------------------------------------------------
        # Blocks D, E (global feature + head)
        # ------------------------------------------------------------------
        with ExitStack() as de:
            zn = de.enter_context(tc.tile_pool(name="zn", bufs=8))
            trash2 = de.enter_context(tc.tile_pool(name="trash2", bufs=2))
            outpool = de.enter_context(tc.tile_pool(name="outp", bufs=1))
            w6 = load_const("w6")
            w7g = load_const("w7g")
            w7x = load_const("w7x")
            w8 = load_const("w8")
            w9 = load_const("w9")
            xs = [x1, x2, x3]

            # conv6: [1024 <- 192] over points
            z6 = []
            s1 = stat.tile([128, 8], f32, tag="s1L")
            s2 = stat.tile([128, 8], f32, tag="s2L")
            for t in range(8):
                ps = psum.tile([128, n], f32, tag="ps")
                for c0, cw in chunks(n):
                    for ci in range(3):
                        nc.tensor.matmul(
                            ps[:, c0:c0 + cw],
                            lhsT=r(w6[:, ci, t * 128:(t + 1) * 128]),
                            rhs=r(xs[ci][:, c0:c0 + cw]),
                            start=(ci == 0), stop=(ci == 2))
                zt = zn.tile([128, n], f32, tag="zn")
                nc.scalar.activation(zt, ps, AF.Copy, accum_out=s1[:, t:t + 1])
                tr = trash2.tile([128, n], f32, tag="trash2")
                nc.scalar.activation(tr, zt, AF.Square, accum_out=s2[:, t:t + 1])
                z6.append(zt)
            a6, bp6 = bn_reduce(s1, s2, 8, CNT1, bn_sb["g6"], bn_sb["b6"], False)
            bn_apply(z6, a6, bp6)

            # g = max over points; y7g[o] = W7g . g
            gmax = persist.tile([128, 8], f32, tag="gmax")
            for t in range(8):
                nc.vector.tensor_reduce(out=gmax[:, t:t + 1], in_=fb(z6[t]),
                                        axis=AX.X, op=ALU.max)
            psg = psum.tile([128, 4], f32, tag="ps")
            for ot in range(4):
                for ct in range(8):
                    nc.tensor.matmul(psg[:, ot:ot + 1],
                                     lhsT=r(w7g[:, ct, ot * 128:(ot + 1) * 128]),
                                     rhs=r(gmax[:, ct:ct + 1]),
                                     start=(ct == 0), stop=(ct == 7))
            y7g = persist.tile([128, 4], f32, tag="y7g")
            nc.scalar.activation(y7g, psg, AF.Copy)

            # conv7: [512 <- 1216] = W7x.[x1;x2;x3] + y7g bias
            z7 = []
            s17 = stat.tile([128, 4], f32, tag="s1L")
            s27 = stat.tile([128, 4], f32, tag="s2L")
            for ot in range(4):
                ps = psum.tile([128, n], f32, tag="ps")
                for c0, cw in chunks(n):
                    for ci in range(3):
                        nc.tensor.matmul(
                            ps[:, c0:c0 + cw],
                            lhsT=r(w7x[:, ci, ot * 128:(ot + 1) * 128]),
                            rhs=r(xs[ci][:, c0:c0 + cw]),
                            start=(ci == 0), stop=(ci == 2))
                zt = zn.tile([128, n], f32, tag="zn")
                ncc = len(list(chunks(n)))
                sub1 = stat.tile([128, ncc], f32, tag="csum")
                for ci, (c0, cw) in enumerate(chunks(n)):
                    nc.vector.tensor_scalar(out=zt[:, c0:c0 + cw],
                                            in0=ps[:, c0:c0 + cw],
                                            scalar1=y7g[:, ot:ot + 1],
                                            scalar2=0.0, op0=ALU.add,
                                            op1=ALU.add,
                                            accum_out=sub1[:, ci:ci + 1])
                nc.vector.tensor_reduce(out=s17[:, ot:ot + 1], in_=sub1,
                                        axis=AX.X, op=ALU.add)
                tr = trash2.tile([128, n], f32, tag="trash2")
                nc.scalar.activation(tr, zt, AF.Square,
                                     accum_out=s27[:, ot:ot + 1])
                z7.append(zt)
            a7, bp7 = bn_reduce(s17, s27, 4, CNT1, bn_sb["g7"], bn_sb["b7"],
                                False)
            bn_apply(z7, a7, bp7)

            # conv8: [256 <- 512]
            z8 = []
            s18 = stat.tile([128, 2], f32, tag="s1L")
            s28 = stat.tile([128, 2], f32, tag="s2L")
            for ot in range(2):
                zt = zn.tile([128, n], f32, tag="zn")
                ncc = len(list(chunks(n)))
                sub1 = stat.tile([128, ncc], f32, tag="csum")
                for ci, (c0, cw) in enumerate(chunks(n)):
                    ps = psum.tile([128, 512], f32, tag="ps")
                    for ct in range(4):
                        nc.tensor.matmul(ps[:, 0:cw],
                                         lhsT=r(w8[:, ct, ot * 128:(ot + 1) * 128]),
                                         rhs=r(z7[ct][:, c0:c0 + cw]),
                                         start=(ct == 0), stop=(ct == 3))
                    nc.scalar.activation(zt[:, c0:c0 + cw], ps[:, 0:cw], AF.Copy,
                                         accum_out=sub1[:, ci:ci + 1])
                nc.vector.tensor_reduce(out=s18[:, ot:ot + 1], in_=sub1,
                                        axis=AX.X, op=ALU.add)
                tr = trash2.tile([128, n], f32, tag="trash2")
                nc.scalar.activation(tr, zt, AF.Square,
                                     accum_out=s28[:, ot:ot + 1])
                z8.append(zt)
            a8, bp8 = bn_reduce(s18, s28, 2, CNT1, bn_sb["g8"], bn_sb["b8"],
                                False)
            bn_apply(z8, a8, bp8)

            # conv9: [4 <- 256], no bn
            outsb = outpool.tile([4, n], f32, tag="outsb")
            for c0, cw in chunks(n):
                ps = psum.tile([128, 512], f32, tag="ps")
                for ct in range(2):
                    nc.tensor.matmul(ps[0:4, 0:cw], lhsT=r(w9[:, ct, :]),
                                     rhs=r(z8[ct][:, c0:c0 + cw]),
                                     start=(ct == 0), stop=(ct == 1))
                nc.scalar.activation(outsb[:, c0:c0 + cw], ps[0:4, 0:cw], AF.Copy)
            nc.sync.dma_start(out=out_ap, in_=outsb)


def build_nc(n=N, k=K, n_cores=NCORES, debug_taps=False):
    import concourse.bacc as bacc
    import concourse.mybir as mybir
    from concourse.tile import TileContext

    f32 = mybir.dt.float32
    f32r = mybir.dt.float32r
    nc = bacc.Bacc("TRN2", target_bir_lowering=False, debug=False,
                   num_devices=n_cores)
    ins = {"x": nc.dram_tensor("x", [6, n], f32, kind="ExternalInput").ap()}
    for nm, shp in WEIGHT_SHAPES.items():
        ins[nm] = nc.dram_tensor(nm, list(shp), f32, kind="ExternalInput").ap()
    out_ap = nc.dram_tensor("out", [4, n], f32, kind="ExternalOutput").ap()
    dbg = None
    if debug_taps:
        dbg = {nm: nc.dram_tensor(nm + "_dbg", [64, n], f32,
                                  kind="ExternalOutput").ap()
               for nm in ["x1", "x2", "x3"]}
    with TileContext(nc) as tc:
        model_body(tc, out_ap, ins, n=n, k=k, n_cores=n_cores, dbg=dbg)
    nc.compile()
    return nc


_NC_CACHE = {}
LAST_RESULT = None


def kernel(x, params):
    global LAST_RESULT
    x = np.asarray(x, np.float32)
    w = prep_weights(params)
    key = (x.shape[2], NCORES)
    if key not in _NC_CACHE:
        _NC_CACHE[key] = build_nc(n=x.shape[2])
    nc = _NC_CACHE[key]
    in_maps = [dict(w, x=np.ascontiguousarray(x[i])) for i in range(x.shape[0])]
    from concourse.bass_utils import run_bass_kernel_spmd
    res = run_bass_kernel_spmd(nc, in_maps, core_ids=list(range(len(in_maps))))
    LAST_RESULT = res
    return np.stack([res.results[i]["out"] for i in range(len(in_maps))], axis=0)
